# revision 38
# baseline (speedup 1.0000x reference)
"""GNN message-passing (segment-mean + 3-layer MLP) Trainium2 kernel.

Strategy (8 NeuronCores, SPMD, full inputs in / full output out):
  - Host: assign nodes to 800 blocks of 64 slots (degree-balanced snake) so
    every block's incoming-edge count fits 6 k-tiles of 128 edges.  Blocks
    0-99 -> core 0, etc.  Edges are bucketed per receiver block, cast to fp8
    at natural scale, and laid out [eslot, ktile*feat] so per-chunk DMAs are
    large and contiguous.  Scatter masks are 64 columns wide (fp8) and carry
    fp8(1/deg(recv)); the fp8 quantization of 1/deg is compensated exactly by
    scaling each edge by 1/(deg*fp8(1/deg)) on the host.
  - Device per core: segment-mean as mask matmuls on the TensorEngine (6
    k-tiles per block accumulated into one 512-col PSUM bank per chunk),
    then the 3-layer MLP over 512/256-node chunks in feature-major layout.
    Everything except PSUM/bias/mask is bf16: halves DMA bytes and enables
    fast-weight-load on the PE (f32r disables FWL).  ~3us of dummy matmuls
    at program start ramp the PE p-state while the DMA pipeline fills.
    Edge/mask slabs stream on the SP HWDGE ring in 2-4 block granules, 4
    chunks deep (the first two chunks are half-sized so compute starts
    early); x/weights/outputs ride the ACT ring, with the three biases
    fused into one transfer and x batched into 4-chunk spans.  Output is
    written bf16 and upcast on the host.
"""
import sys

sys.path.insert(0, "/opt/trn_rl_repo")

import numpy as np
import ml_dtypes

from concourse import bacc
import concourse.mybir as mybir
import concourse.tile as tile
from concourse.bass_utils import run_bass_kernel_spmd

# problem shape (hardcoded per contract)
N_NODES = 50000
N_EDGES = 600000
D = 128          # node/edge feature dim
DH = 512         # hidden dim
C = 8            # cores
W = 64           # node slots per block
BPC = 100        # node blocks per core
NB = C * BPC     # 800 blocks total
SLOTS = BPC * W  # 6400 node slots per core
T_BLK = 6        # edge k-tiles (128 edges) per block
TT = BPC * T_BLK   # k-tiles per core
CHUNKS = [4] * 2 + [8] * 11 + [4]  # blocks per MLP chunk (256-node ramp start)
PREF = 4         # chunks of edge-slab prefetch depth

F32 = mybir.dt.float32
BF16 = mybir.dt.bfloat16
FP8 = mybir.dt.float8e4

_prog_cache = {}
LAST_RESULTS = None  # BassKernelResults of the most recent run (for test.py)


def _build_program(t_blk=T_BLK):
    if t_blk in _prog_cache:
        return _prog_cache[t_blk]
    tt = BPC * t_blk
    nc = bacc.Bacc("TRN2", target_bir_lowering=False)

    DW = D + W  # fused edge-feature + mask row stride per k-tile
    xT_d = nc.declare_dram_parameter("xT", [128, SLOTS], BF16, isOutput=False)
    em_d = nc.declare_dram_parameter("em", [128, tt * DW], FP8, isOutput=False)
    w1_d = nc.declare_dram_parameter("w1", [2 * D, DH], BF16, isOutput=False)
    w2_d = nc.declare_dram_parameter("w2", [DH, DH], BF16, isOutput=False)
    w3_d = nc.declare_dram_parameter("w3", [DH, D], BF16, isOutput=False)
    bb_d = nc.declare_dram_parameter("bb", [128, 9], F32, isOutput=False)
    out_d = nc.declare_dram_parameter("outT", [128, SLOTS], BF16, isOutput=True)

    RELU = mybir.ActivationFunctionType.Relu
    ADD = mybir.AluOpType.add

    n_chunks = len(CHUNKS)
    chunk_blk0 = [0]
    for c_i in range(1, n_chunks):
        chunk_blk0.append(chunk_blk0[-1] + CHUNKS[c_i - 1])

    with tile.TileContext(nc) as tc:
        with (
            tc.tile_pool(name="pers", bufs=1) as pers,
            tc.tile_pool(name="eap", bufs=14) as eap,
            tc.tile_pool(name="xp", bufs=4) as xp,
            tc.tile_pool(name="actp", bufs=4) as actp,
            tc.tile_pool(name="scat_ps", bufs=2, space="PSUM") as scat_ps,
            tc.tile_pool(name="mlp_ps", bufs=6, space="PSUM") as mlp_ps,
        ):
            # granules[(chunk, block)] = (em_tile, local_block)
            gran = {}

            def load_granule(c_i, b0, nblk):
                kt0 = (chunk_blk0[c_i] + b0) * t_blk
                nkt = nblk * t_blk
                em_t = eap.tile([128, 4 * t_blk * DW], FP8, tag="em")
                nc.sync.dma_start(
                    out=em_t[:, : nkt * DW], in_=em_d[:, kt0 * DW : (kt0 + nkt) * DW]
                )
                for lb in range(nblk):
                    gran[(c_i, b0 + lb)] = (em_t, lb)

            def load_chunk_slabs(c_i, granule_blks):
                nb = CHUNKS[c_i]
                if isinstance(granule_blks, int):
                    granule_blks = [granule_blks] * ((nb + granule_blks - 1) // granule_blks)
                b0 = 0
                for g in granule_blks:
                    g = min(g, nb - b0)
                    if g <= 0:
                        break
                    load_granule(c_i, b0, g)
                    b0 += g

            x_ts = {}

            def load_x_span(c_lo, c_hi):
                col_lo = chunk_blk0[c_lo] * W
                col_hi = chunk_blk0[c_hi] * W + CHUNKS[c_hi] * W
                xt = xp.tile([128, 2560], BF16, tag="x")
                nc.scalar.dma_start(out=xt[:, : col_hi - col_lo], in_=xT_d[:, col_lo:col_hi])
                for c in range(c_lo, c_hi + 1):
                    x_ts[c] = (xt, chunk_blk0[c] * W - col_lo)

            # --- PE p-state warmup: dummy matmuls with no deps run during
            # the DMA pipeline fill, so the real scatter/MLP start at the
            # full 2.4 GHz p-state instead of ramping through it.  Memsets
            # ride the otherwise-idle GpSimd queue. ---
            warm_in = pers.tile([128, 128], BF16)
            nc.gpsimd.memset(warm_in[:], 1.0)
            warm_rhs = pers.tile([128, 256], BF16)
            nc.gpsimd.memset(warm_rhs[:], 1.0)
            warm_ps = scat_ps.tile([128, 512], F32, tag="scat")
            for wi in range(13):
                nc.tensor.matmul(
                    out=warm_ps[:, :256],
                    lhsT=warm_in[:],
                    rhs=warm_rhs[:],
                    start=(wi == 0),
                    stop=(wi == 12),
                )

            # --- fused edge+mask slabs stream on the SP ring.  Small first
            # granules: the DMA rings allow ~3 transfers in flight sharing
            # bandwidth round-robin, so chunk 0's data must not ride behind
            # megabyte prefetches. ---
            load_chunk_slabs(0, 1)
            # --- x + persistent tiles on the ACT ring (first x span and w1
            # gate the first L1; fused biases ride one transfer; the big x
            # spans are issued from inside the loop so they queue behind the
            # early chunks' relus instead of starving the ramp) ---
            load_x_span(0, 1)
            w1t = pers.tile([128, 2, DH], BF16)
            nc.scalar.dma_start(out=w1t[:], in_=w1_d[:].rearrange("(k p) m -> p k m", p=128))
            bbt = pers.tile([128, 9], F32)
            nc.scalar.dma_start(out=bbt[:], in_=bb_d[:])
            load_chunk_slabs(1, 2)
            w2t = pers.tile([128, 4, DH], BF16)
            nc.scalar.dma_start(out=w2t[:], in_=w2_d[:].rearrange("(k p) m -> p k m", p=128))
            load_chunk_slabs(2, 4)
            w3t = pers.tile([128, 4, D], BF16)
            nc.scalar.dma_start(out=w3t[:], in_=w3_d[:].rearrange("(k p) m -> p k m", p=128))
            load_chunk_slabs(3, 4)

            def emit_scatter(c_i, b_lo, b_hi, ps):
                for b in range(b_lo, b_hi):
                    em_t, lb = gran.pop((c_i, b))
                    for t in range(t_blk):
                        j = (lb * t_blk + t)
                        nc.tensor.matmul(
                            out=ps[:, b * W : (b + 1) * W],
                            lhsT=em_t[:, j * DW : j * DW + D],
                            rhs=em_t[:, j * DW + D : (j + 1) * DW],
                            start=(t == 0),
                            stop=(t == t_blk - 1),
                        )

            def emit_l1(c_i, mean_t, h1_t):
                NCW = CHUNKS[c_i] * W
                xt_c, xoff = x_ts[c_i]
                pms = []
                for m in range(4):
                    pm = mlp_ps.tile([128, 512], F32, tag="mlp")
                    nc.tensor.matmul(
                        out=pm[:, :NCW],
                        lhsT=w1t[:, 0, m * 128 : (m + 1) * 128],
                        rhs=xt_c[:, xoff : xoff + NCW],
                        start=True,
                        stop=False,
                    )
                    pms.append(pm)
                for m in range(4):
                    nc.tensor.matmul(
                        out=pms[m][:, :NCW],
                        lhsT=w1t[:, 1, m * 128 : (m + 1) * 128],
                        rhs=mean_t[:, :NCW],
                        start=False,
                        stop=True,
                    )
                    if m % 2 == 0:
                        nc.scalar.activation(
                            out=h1_t[:, m, :NCW], in_=pms[m][:, :NCW], func=RELU, bias=bbt[:, m : m + 1]
                        )
                    else:
                        nc.vector.tensor_scalar(
                            out=h1_t[:, m, :NCW], in0=pms[m][:, :NCW], scalar1=bbt[:, m : m + 1],
                            scalar2=0.0, op0=ADD, op1=mybir.AluOpType.max,
                        )

            def emit_l2(c_i, h1_t, h2_t):
                NCW = CHUNKS[c_i] * W
                for m in range(4):
                    pm = mlp_ps.tile([128, 512], F32, tag="mlp")
                    for k in range(4):
                        nc.tensor.matmul(
                            out=pm[:, :NCW],
                            lhsT=w2t[:, k, m * 128 : (m + 1) * 128],
                            rhs=h1_t[:, k, :NCW],
                            start=(k == 0),
                            stop=(k == 3),
                        )
                    if m % 2 == 0:
                        nc.scalar.activation(
                            out=h2_t[:, m, :NCW], in_=pm[:, :NCW], func=RELU, bias=bbt[:, 4 + m : 5 + m]
                        )
                    else:
                        nc.vector.tensor_scalar(
                            out=h2_t[:, m, :NCW], in0=pm[:, :NCW], scalar1=bbt[:, 4 + m : 5 + m],
                            scalar2=0.0, op0=ADD, op1=mybir.AluOpType.max,
                        )

            def emit_l3(c_i, h2_t, col0, split_bias=False):
                NCW = CHUNKS[c_i] * W
                pm = mlp_ps.tile([128, 512], F32, tag="mlp")
                for k in range(4):
                    nc.tensor.matmul(
                        out=pm[:, :NCW],
                        lhsT=w3t[:, k, :],
                        rhs=h2_t[:, k, :NCW],
                        start=(k == 0),
                        stop=(k == 3),
                    )
                out_t = actp.tile([128, 512], BF16, tag="out")
                if split_bias:
                    # final chunk: halve the bias-add latency by splitting it
                    # across the DVE and ACT engines (it gates the last DMA)
                    h = NCW // 2
                    nc.vector.tensor_scalar_add(out_t[:, :h], pm[:, :h], bbt[:, 8:9])
                    nc.scalar.activation(
                        out=out_t[:, h:NCW], in_=pm[:, h:NCW],
                        func=mybir.ActivationFunctionType.Identity, bias=bbt[:, 8:9],
                    )
                else:
                    nc.vector.tensor_scalar_add(out_t[:, :NCW], pm[:, :NCW], bbt[:, 8:9])
                # out DMAs ride the otherwise-idle GpSimd queue so their
                # ~0.6us descriptor-generation never delays ACT relus
                nc.gpsimd.dma_start(out=out_d[:, col0 : col0 + NCW], in_=out_t[:, :NCW])

            # chunk groups: solo during the DMA-bound ramp, pairs afterward
            # (deeper groups oversubscribe the 6 MLP PSUM banks: 9 allocs
            # per chunk outrun the relu retirements and every L1 matmul
            # waits on a bank).  Pairing keeps the MLP matmul stream
            # contiguous (scatter A, scatter B, then L1A L1B L2A L2B L3A
            # L3B): the PSUM->SBUF mean copy and the relu latencies hide
            # inside the MLP stream instead of costing a PE pipeline/
            # weight-load break per transition.
            groups = [[0], [1]] + [[c, c + 1] for c in range(2, n_chunks - 1, 2)]
            chunk_col0 = {}
            acc = 0
            for c_i, nb in enumerate(CHUNKS):
                chunk_col0[c_i] = acc
                acc += nb * W

            for g in groups:
                means, h1s = {}, {}
                for c_i in g:
                    ps = scat_ps.tile([128, 512], F32, tag="scat")
                    emit_scatter(c_i, 0, CHUNKS[c_i], ps)
                    NCW = CHUNKS[c_i] * W
                    means[c_i] = actp.tile([128, 512], BF16, tag="mean", name="mean_t")
                    if len(g) == 1:
                        # ramp solo: the mean copy gates L1 directly — halve
                        # its latency by splitting across DVE and ACT
                        h = NCW // 2
                        nc.vector.tensor_copy(out=means[c_i][:, :h], in_=ps[:, :h])
                        nc.scalar.activation(
                            out=means[c_i][:, h:NCW], in_=ps[:, h:NCW],
                            func=mybir.ActivationFunctionType.Copy,
                        )
                    else:
                        nc.vector.tensor_copy(out=means[c_i][:, :NCW], in_=ps[:, :NCW])
                    if c_i + PREF < n_chunks:
                        load_chunk_slabs(c_i + PREF, 4)
                for c_i in g:
                    h1s[c_i] = actp.tile([128, 4, 512], BF16, tag="h1", name="h1_t")
                    emit_l1(c_i, means[c_i], h1s[c_i])
                # deferred x spans: issued here so the DMA queues behind the
                # early chunks' relus on the ACT ring (ramp bandwidth stays
                # with chunk 0-1 data)
                if g[0] == 0:
                    load_x_span(2, 6)
                elif g[0] == 2:
                    load_x_span(7, 11)
                elif g[0] == 6:
                    load_x_span(12, n_chunks - 1)
                h2s = {}
                for c_i in g:
                    h2s[c_i] = actp.tile([128, 4, 512], BF16, tag="h2", name="h2_t")
                    emit_l2(c_i, h1s[c_i], h2s[c_i])
                # in the final group, emit the small trailing chunk's L3
                # first so the last bias+DMA chain hangs off the bigger
                # chunk, whose bias is split across two engines
                l3_order = list(g) if g is not groups[-1] else list(reversed(g))
                for j, c_i in enumerate(l3_order):
                    final = g is groups[-1] and j == len(l3_order) - 1
                    emit_l3(c_i, h2s[c_i], chunk_col0[c_i], split_bias=final)

    nc.compile()
    _prog_cache[t_blk] = nc
    return nc


def _preprocess(x, edge_index, edge_attr):
    recv = np.asarray(edge_index)[1].astype(np.int64)
    deg = np.bincount(recv, minlength=N_NODES)
    # snake assignment of degree-sorted nodes into NB blocks (62-63 nodes each)
    order = np.argsort(-deg, kind="stable")
    i = np.arange(N_NODES)
    rnd, pos = i // NB, i % NB
    blk = np.where(rnd % 2 == 0, pos, NB - 1 - pos)
    node_block = np.empty(N_NODES, np.int64)
    node_slot = np.empty(N_NODES, np.int64)
    node_block[order] = blk
    node_slot[order] = rnd
    node_core = node_block // BPC
    node_col = (node_block % BPC) * W + node_slot

    eb = node_block[recv]
    bc = np.bincount(eb, minlength=NB)
    t_blk = max(T_BLK, int(-(-int(bc.max()) // 128)))  # >= ceil(max_load/128)
    tt = BPC * t_blk

    eorder = np.argsort(eb, kind="stable")
    eb_s = eb[eorder]
    starts = np.zeros(NB, np.int64)
    starts[1:] = np.cumsum(bc)[:-1]
    ewithin = np.arange(N_EDGES) - starts[eb_s]
    ktile = ewithin // 128
    eslot = ewithin % 128
    ecore = eb_s // BPC
    kt_in_core = (eb_s % BPC) * t_blk + ktile

    # fp8 edges at natural N(0,1) scale; 1/deg rides in the mask as fp8(1/deg)
    # with the fp8-quantization of 1/deg compensated exactly on the edge side:
    # alpha = 1/(deg * fp8(1/deg)) so mask * alpha * sum == true mean
    degc = np.maximum(deg, 1).astype(np.float32)
    mval = (1.0 / degc).astype(ml_dtypes.float8_e4m3)
    alpha = 1.0 / (degc * mval.astype(np.float32))
    ea_scaled = np.asarray(edge_attr, np.float32) * alpha[recv][:, None]
    ea_f8 = ea_scaled.astype(ml_dtypes.float8_e4m3)
    # fused [edge-feature | mask] rows: one slab DMA per granule on-device.
    # masks carry fp8(1/deg) in the receiver's slot column.
    DW = D + W
    em_buf = np.zeros((C, tt, 128, DW), ml_dtypes.float8_e4m3)
    em_buf[ecore, kt_in_core, eslot, :D] = ea_f8[eorder]
    em_buf[ecore, kt_in_core, eslot, D + (node_col[recv] % W)[eorder]] = mval[recv][eorder]

    X_all = np.zeros((C, SLOTS, D), ml_dtypes.bfloat16)
    X_all[node_core, node_col] = np.asarray(x, np.float32).astype(ml_dtypes.bfloat16)

    shards = []
    for c in range(C):
        shards.append(
            dict(
                xT=np.ascontiguousarray(X_all[c].T),
                em=np.ascontiguousarray(em_buf[c].transpose(1, 0, 2).reshape(128, tt * DW)),
            )
        )
    return shards, node_core, node_col, t_blk


def kernel(x, edge_index, edge_attr, W1, b1, W2, b2, W3, b3, _trace=False):
    global LAST_RESULTS
    shards, node_core, node_col, t_blk = _preprocess(x, edge_index, edge_attr)

    W1 = np.ascontiguousarray(np.asarray(W1, np.float32).astype(ml_dtypes.bfloat16))
    W2 = np.ascontiguousarray(np.asarray(W2, np.float32).astype(ml_dtypes.bfloat16))
    W3 = np.ascontiguousarray(np.asarray(W3, np.float32).astype(ml_dtypes.bfloat16))
    bb = np.concatenate(
        [
            np.asarray(b1, np.float32).reshape(4, 128).T,
            np.asarray(b2, np.float32).reshape(4, 128).T,
            np.asarray(b3, np.float32).reshape(1, 128).T,
        ],
        axis=1,
    )
    bb = np.ascontiguousarray(bb)

    in_maps = []
    for c in range(C):
        m = dict(shards[c])
        m.update(w1=W1, w2=W2, w3=W3, bb=bb)
        in_maps.append(m)

    nc = _build_program(t_blk)
    res = run_bass_kernel_spmd(nc, in_maps, core_ids=list(range(C)), trace=_trace)
    LAST_RESULTS = res

    outs = np.stack([res.results[c]["outT"] for c in range(C)])  # [C, 128, SLOTS] bf16
    out = outs.transpose(0, 2, 1)[node_core, node_col]
    return np.ascontiguousarray(out, dtype=np.float32)



# revision 41
# speedup vs baseline: 1.0023x; 1.0023x over previous
"""GNN message-passing (segment-mean + 3-layer MLP) Trainium2 kernel.

Strategy (8 NeuronCores, SPMD, full inputs in / full output out):
  - Host: assign nodes to 800 blocks of 64 slots (degree-balanced snake) so
    every block's incoming-edge count fits 6 k-tiles of 128 edges.  Blocks
    0-99 -> core 0, etc.  Edges are bucketed per receiver block, cast to fp8
    at natural scale, and laid out [eslot, ktile*feat] so per-chunk DMAs are
    large and contiguous.  Scatter masks are 64 columns wide (fp8) and carry
    fp8(1/deg(recv)); the fp8 quantization of 1/deg is compensated exactly by
    scaling each edge by 1/(deg*fp8(1/deg)) on the host.
  - Device per core: segment-mean as mask matmuls on the TensorEngine (6
    k-tiles per block accumulated into one 512-col PSUM bank per chunk),
    then the 3-layer MLP over 512/256-node chunks in feature-major layout.
    Everything except PSUM/bias/mask is bf16: halves DMA bytes and enables
    fast-weight-load on the PE (f32r disables FWL).  ~3us of dummy matmuls
    at program start ramp the PE p-state while the DMA pipeline fills.
    Edge/mask slabs stream on the SP HWDGE ring in 2-4 block granules, 4
    chunks deep (the first two chunks are half-sized so compute starts
    early); x/weights/outputs ride the ACT ring, with the three biases
    fused into one transfer and x batched into 4-chunk spans.  Output is
    written bf16 and upcast on the host.
"""
import sys

sys.path.insert(0, "/opt/trn_rl_repo")

import numpy as np
import ml_dtypes

from concourse import bacc
import concourse.mybir as mybir
import concourse.tile as tile
from concourse.bass_utils import run_bass_kernel_spmd

# problem shape (hardcoded per contract)
N_NODES = 50000
N_EDGES = 600000
D = 128          # node/edge feature dim
DH = 512         # hidden dim
C = 8            # cores
W = 64           # node slots per block
BPC = 100        # node blocks per core
NB = C * BPC     # 800 blocks total
SLOTS = BPC * W  # 6400 node slots per core
T_BLK = 6        # edge k-tiles (128 edges) per block
TT = BPC * T_BLK   # k-tiles per core
CHUNKS = [4] * 2 + [8] * 11 + [4]  # blocks per MLP chunk (256-node ramp start)
PREF = 4         # chunks of edge-slab prefetch depth

F32 = mybir.dt.float32
BF16 = mybir.dt.bfloat16
FP8 = mybir.dt.float8e4

_prog_cache = {}
LAST_RESULTS = None  # BassKernelResults of the most recent run (for test.py)


def _build_program(t_blk=T_BLK):
    if t_blk in _prog_cache:
        return _prog_cache[t_blk]
    tt = BPC * t_blk
    nc = bacc.Bacc("TRN2", target_bir_lowering=False)

    DW = D + W  # fused edge-feature + mask row stride per k-tile
    xT_d = nc.declare_dram_parameter("xT", [128, SLOTS], BF16, isOutput=False)
    em_d = nc.declare_dram_parameter("em", [128, tt * DW], FP8, isOutput=False)
    w1_d = nc.declare_dram_parameter("w1", [2 * D, DH], BF16, isOutput=False)
    w2_d = nc.declare_dram_parameter("w2", [DH, DH], BF16, isOutput=False)
    w3_d = nc.declare_dram_parameter("w3", [DH, D], BF16, isOutput=False)
    bb_d = nc.declare_dram_parameter("bb", [128, 9], F32, isOutput=False)
    out_d = nc.declare_dram_parameter("outT", [128, SLOTS], BF16, isOutput=True)

    RELU = mybir.ActivationFunctionType.Relu
    ADD = mybir.AluOpType.add

    n_chunks = len(CHUNKS)
    chunk_blk0 = [0]
    for c_i in range(1, n_chunks):
        chunk_blk0.append(chunk_blk0[-1] + CHUNKS[c_i - 1])

    with tile.TileContext(nc) as tc:
        with (
            tc.tile_pool(name="pers", bufs=1) as pers,
            tc.tile_pool(name="eap", bufs=14) as eap,
            tc.tile_pool(name="xp", bufs=4) as xp,
            tc.tile_pool(name="actp", bufs=4) as actp,
            tc.tile_pool(name="scat_ps", bufs=2, space="PSUM") as scat_ps,
            tc.tile_pool(name="mlp_ps", bufs=6, space="PSUM") as mlp_ps,
        ):
            # granules[(chunk, block)] = (em_tile, local_block)
            gran = {}

            def load_granule(c_i, b0, nblk):
                kt0 = (chunk_blk0[c_i] + b0) * t_blk
                nkt = nblk * t_blk
                em_t = eap.tile([128, 4 * t_blk * DW], FP8, tag="em")
                nc.sync.dma_start(
                    out=em_t[:, : nkt * DW], in_=em_d[:, kt0 * DW : (kt0 + nkt) * DW]
                )
                for lb in range(nblk):
                    gran[(c_i, b0 + lb)] = (em_t, lb)

            def load_chunk_slabs(c_i, granule_blks):
                nb = CHUNKS[c_i]
                if isinstance(granule_blks, int):
                    granule_blks = [granule_blks] * ((nb + granule_blks - 1) // granule_blks)
                b0 = 0
                for g in granule_blks:
                    g = min(g, nb - b0)
                    if g <= 0:
                        break
                    load_granule(c_i, b0, g)
                    b0 += g

            x_ts = {}

            def load_x_span(c_lo, c_hi):
                col_lo = chunk_blk0[c_lo] * W
                col_hi = chunk_blk0[c_hi] * W + CHUNKS[c_hi] * W
                xt = xp.tile([128, 2560], BF16, tag="x")
                nc.scalar.dma_start(out=xt[:, : col_hi - col_lo], in_=xT_d[:, col_lo:col_hi])
                for c in range(c_lo, c_hi + 1):
                    x_ts[c] = (xt, chunk_blk0[c] * W - col_lo)

            # --- PE p-state warmup: dummy matmuls with no deps run during
            # the DMA pipeline fill, so the real scatter/MLP start at the
            # full 2.4 GHz p-state instead of ramping through it.  Memsets
            # ride the otherwise-idle GpSimd queue. ---
            warm_in = pers.tile([128, 128], BF16)
            nc.gpsimd.memset(warm_in[:], 1.0)
            warm_rhs = pers.tile([128, 256], BF16)
            nc.gpsimd.memset(warm_rhs[:], 1.0)
            warm_ps = scat_ps.tile([128, 512], F32, tag="scat")
            for wi in range(13):
                nc.tensor.matmul(
                    out=warm_ps[:, :256],
                    lhsT=warm_in[:],
                    rhs=warm_rhs[:],
                    start=(wi == 0),
                    stop=(wi == 12),
                )

            # --- fused edge+mask slabs stream on the SP ring.  Small first
            # granules: the DMA rings allow ~3 transfers in flight sharing
            # bandwidth round-robin, so chunk 0's data must not ride behind
            # megabyte prefetches. ---
            load_chunk_slabs(0, 1)
            # --- x + persistent tiles on the ACT ring (first x span and w1
            # gate the first L1; fused biases ride one transfer; the big x
            # spans are issued from inside the loop so they queue behind the
            # early chunks' relus instead of starving the ramp) ---
            load_x_span(0, 1)
            w1t = pers.tile([128, 2, DH], BF16)
            nc.scalar.dma_start(out=w1t[:], in_=w1_d[:].rearrange("(k p) m -> p k m", p=128))
            bbt = pers.tile([128, 9], F32)
            nc.scalar.dma_start(out=bbt[:], in_=bb_d[:])
            load_chunk_slabs(1, 2)
            w2t = pers.tile([128, 4, DH], BF16)
            nc.scalar.dma_start(out=w2t[:], in_=w2_d[:].rearrange("(k p) m -> p k m", p=128))
            load_chunk_slabs(2, 4)
            w3t = pers.tile([128, 4, D], BF16)
            nc.scalar.dma_start(out=w3t[:], in_=w3_d[:].rearrange("(k p) m -> p k m", p=128))
            load_chunk_slabs(3, 4)

            def emit_scatter(c_i, b_lo, b_hi, ps):
                for b in range(b_lo, b_hi):
                    em_t, lb = gran.pop((c_i, b))
                    for t in range(t_blk):
                        j = (lb * t_blk + t)
                        nc.tensor.matmul(
                            out=ps[:, b * W : (b + 1) * W],
                            lhsT=em_t[:, j * DW : j * DW + D],
                            rhs=em_t[:, j * DW + D : (j + 1) * DW],
                            start=(t == 0),
                            stop=(t == t_blk - 1),
                        )

            def emit_l1(c_i, mean_t, h1_t):
                NCW = CHUNKS[c_i] * W
                xt_c, xoff = x_ts[c_i]
                pms = []
                for m in range(4):
                    pm = mlp_ps.tile([128, 512], F32, tag="mlp")
                    nc.tensor.matmul(
                        out=pm[:, :NCW],
                        lhsT=w1t[:, 0, m * 128 : (m + 1) * 128],
                        rhs=xt_c[:, xoff : xoff + NCW],
                        start=True,
                        stop=False,
                    )
                    pms.append(pm)
                for m in range(4):
                    nc.tensor.matmul(
                        out=pms[m][:, :NCW],
                        lhsT=w1t[:, 1, m * 128 : (m + 1) * 128],
                        rhs=mean_t[:, :NCW],
                        start=False,
                        stop=True,
                    )
                    if m % 2 == 0:
                        nc.scalar.activation(
                            out=h1_t[:, m, :NCW], in_=pms[m][:, :NCW], func=RELU, bias=bbt[:, m : m + 1]
                        )
                    else:
                        nc.vector.tensor_scalar(
                            out=h1_t[:, m, :NCW], in0=pms[m][:, :NCW], scalar1=bbt[:, m : m + 1],
                            scalar2=0.0, op0=ADD, op1=mybir.AluOpType.max,
                        )

            def emit_l2(c_i, h1_t, h2_t):
                NCW = CHUNKS[c_i] * W
                for m in range(4):
                    pm = mlp_ps.tile([128, 512], F32, tag="mlp")
                    for k in range(4):
                        nc.tensor.matmul(
                            out=pm[:, :NCW],
                            lhsT=w2t[:, k, m * 128 : (m + 1) * 128],
                            rhs=h1_t[:, k, :NCW],
                            start=(k == 0),
                            stop=(k == 3),
                        )
                    if m % 2 == 0:
                        nc.scalar.activation(
                            out=h2_t[:, m, :NCW], in_=pm[:, :NCW], func=RELU, bias=bbt[:, 4 + m : 5 + m]
                        )
                    else:
                        nc.vector.tensor_scalar(
                            out=h2_t[:, m, :NCW], in0=pm[:, :NCW], scalar1=bbt[:, 4 + m : 5 + m],
                            scalar2=0.0, op0=ADD, op1=mybir.AluOpType.max,
                        )

            def emit_l3_psum(c_i, h2_t):
                NCW = CHUNKS[c_i] * W
                pm = mlp_ps.tile([128, 512], F32, tag="mlp")
                for k in range(4):
                    nc.tensor.matmul(
                        out=pm[:, :NCW],
                        lhsT=w3t[:, k, :],
                        rhs=h2_t[:, k, :NCW],
                        start=(k == 0),
                        stop=(k == 3),
                    )
                return pm

            def emit_l3(c_i, h2_t, col0):
                NCW = CHUNKS[c_i] * W
                pm = emit_l3_psum(c_i, h2_t)
                out_t = actp.tile([128, 512], BF16, tag="out")
                nc.vector.tensor_scalar_add(out_t[:, :NCW], pm[:, :NCW], bbt[:, 8:9])
                # out DMAs ride the otherwise-idle GpSimd queue so their
                # ~0.6us descriptor-generation never delays ACT relus
                nc.gpsimd.dma_start(out=out_d[:, col0 : col0 + NCW], in_=out_t[:, :NCW])

            # chunk groups: solo during the DMA-bound ramp, pairs afterward
            # (deeper groups oversubscribe the 6 MLP PSUM banks: 9 allocs
            # per chunk outrun the relu retirements and every L1 matmul
            # waits on a bank).  Pairing keeps the MLP matmul stream
            # contiguous (scatter A, scatter B, then L1A L1B L2A L2B L3A
            # L3B): the PSUM->SBUF mean copy and the relu latencies hide
            # inside the MLP stream instead of costing a PE pipeline/
            # weight-load break per transition.
            groups = [[0], [1]] + [[c, c + 1] for c in range(2, n_chunks - 1, 2)]
            chunk_col0 = {}
            acc = 0
            for c_i, nb in enumerate(CHUNKS):
                chunk_col0[c_i] = acc
                acc += nb * W

            for g in groups:
                means, h1s = {}, {}
                for c_i in g:
                    ps = scat_ps.tile([128, 512], F32, tag="scat")
                    emit_scatter(c_i, 0, CHUNKS[c_i], ps)
                    NCW = CHUNKS[c_i] * W
                    means[c_i] = actp.tile([128, 512], BF16, tag="mean", name="mean_t")
                    if len(g) == 1:
                        # ramp solo: the mean copy gates L1 directly — halve
                        # its latency by splitting across DVE and ACT
                        h = NCW // 2
                        nc.vector.tensor_copy(out=means[c_i][:, :h], in_=ps[:, :h])
                        nc.scalar.activation(
                            out=means[c_i][:, h:NCW], in_=ps[:, h:NCW],
                            func=mybir.ActivationFunctionType.Copy,
                        )
                    else:
                        nc.vector.tensor_copy(out=means[c_i][:, :NCW], in_=ps[:, :NCW])
                    if c_i + PREF < n_chunks:
                        load_chunk_slabs(c_i + PREF, 4)
                for c_i in g:
                    h1s[c_i] = actp.tile([128, 4, 512], BF16, tag="h1", name="h1_t")
                    emit_l1(c_i, means[c_i], h1s[c_i])
                # deferred x spans: issued here so the DMA queues behind the
                # early chunks' relus on the ACT ring (ramp bandwidth stays
                # with chunk 0-1 data)
                if g[0] == 0:
                    load_x_span(2, 6)
                elif g[0] == 2:
                    load_x_span(7, 11)
                elif g[0] == 6:
                    load_x_span(12, n_chunks - 1)
                h2s = {}
                for c_i in g:
                    h2s[c_i] = actp.tile([128, 4, 512], BF16, tag="h2", name="h2_t")
                    emit_l2(c_i, h1s[c_i], h2s[c_i])
                if g is not groups[-1]:
                    for c_i in g:
                        emit_l3(c_i, h2s[c_i], chunk_col0[c_i])
                else:
                    # final pair: small chunk's L3 first, bias of the last
                    # chunk split across DVE+ACT, both outputs fused into
                    # ONE tile and ONE DMA issued from the Vector queue
                    # (empty ring -> short drain at program teardown)
                    cA, cB = g[0], g[1]
                    wA, wB = CHUNKS[cA] * W, CHUNKS[cB] * W
                    out_f = actp.tile([128, wA + wB], BF16, tag="out", name="out_t")
                    pmB = emit_l3_psum(cB, h2s[cB])
                    nc.vector.tensor_scalar_add(out_f[:, wA : wA + wB], pmB[:, :wB], bbt[:, 8:9])
                    pmA = emit_l3_psum(cA, h2s[cA])
                    h = wA // 2
                    nc.vector.tensor_scalar_add(out_f[:, :h], pmA[:, :h], bbt[:, 8:9])
                    nc.scalar.activation(
                        out=out_f[:, h:wA], in_=pmA[:, h:wA],
                        func=mybir.ActivationFunctionType.Identity, bias=bbt[:, 8:9],
                    )
                    nc.scalar.dma_start(
                        out=out_d[:, chunk_col0[cA] : chunk_col0[cA] + wA + wB],
                        in_=out_f[:, : wA + wB],
                    )

    nc.compile()
    _prog_cache[t_blk] = nc
    return nc


def _preprocess(x, edge_index, edge_attr):
    recv = np.asarray(edge_index)[1].astype(np.int64)
    deg = np.bincount(recv, minlength=N_NODES)
    # snake assignment of degree-sorted nodes into NB blocks (62-63 nodes each)
    order = np.argsort(-deg, kind="stable")
    i = np.arange(N_NODES)
    rnd, pos = i // NB, i % NB
    blk = np.where(rnd % 2 == 0, pos, NB - 1 - pos)
    node_block = np.empty(N_NODES, np.int64)
    node_slot = np.empty(N_NODES, np.int64)
    node_block[order] = blk
    node_slot[order] = rnd
    node_core = node_block // BPC
    node_col = (node_block % BPC) * W + node_slot

    eb = node_block[recv]
    bc = np.bincount(eb, minlength=NB)
    t_blk = max(T_BLK, int(-(-int(bc.max()) // 128)))  # >= ceil(max_load/128)
    tt = BPC * t_blk

    eorder = np.argsort(eb, kind="stable")
    eb_s = eb[eorder]
    starts = np.zeros(NB, np.int64)
    starts[1:] = np.cumsum(bc)[:-1]
    ewithin = np.arange(N_EDGES) - starts[eb_s]
    ktile = ewithin // 128
    eslot = ewithin % 128
    ecore = eb_s // BPC
    kt_in_core = (eb_s % BPC) * t_blk + ktile

    # fp8 edges at natural N(0,1) scale; 1/deg rides in the mask as fp8(1/deg)
    # with the fp8-quantization of 1/deg compensated exactly on the edge side:
    # alpha = 1/(deg * fp8(1/deg)) so mask * alpha * sum == true mean
    degc = np.maximum(deg, 1).astype(np.float32)
    mval = (1.0 / degc).astype(ml_dtypes.float8_e4m3)
    alpha = 1.0 / (degc * mval.astype(np.float32))
    ea_scaled = np.asarray(edge_attr, np.float32) * alpha[recv][:, None]
    ea_f8 = ea_scaled.astype(ml_dtypes.float8_e4m3)
    # fused [edge-feature | mask] rows: one slab DMA per granule on-device.
    # masks carry fp8(1/deg) in the receiver's slot column.
    DW = D + W
    em_buf = np.zeros((C, tt, 128, DW), ml_dtypes.float8_e4m3)
    em_buf[ecore, kt_in_core, eslot, :D] = ea_f8[eorder]
    em_buf[ecore, kt_in_core, eslot, D + (node_col[recv] % W)[eorder]] = mval[recv][eorder]

    X_all = np.zeros((C, SLOTS, D), ml_dtypes.bfloat16)
    X_all[node_core, node_col] = np.asarray(x, np.float32).astype(ml_dtypes.bfloat16)

    shards = []
    for c in range(C):
        shards.append(
            dict(
                xT=np.ascontiguousarray(X_all[c].T),
                em=np.ascontiguousarray(em_buf[c].transpose(1, 0, 2).reshape(128, tt * DW)),
            )
        )
    return shards, node_core, node_col, t_blk


def kernel(x, edge_index, edge_attr, W1, b1, W2, b2, W3, b3, _trace=False):
    global LAST_RESULTS
    shards, node_core, node_col, t_blk = _preprocess(x, edge_index, edge_attr)

    W1 = np.ascontiguousarray(np.asarray(W1, np.float32).astype(ml_dtypes.bfloat16))
    W2 = np.ascontiguousarray(np.asarray(W2, np.float32).astype(ml_dtypes.bfloat16))
    W3 = np.ascontiguousarray(np.asarray(W3, np.float32).astype(ml_dtypes.bfloat16))
    bb = np.concatenate(
        [
            np.asarray(b1, np.float32).reshape(4, 128).T,
            np.asarray(b2, np.float32).reshape(4, 128).T,
            np.asarray(b3, np.float32).reshape(1, 128).T,
        ],
        axis=1,
    )
    bb = np.ascontiguousarray(bb)

    in_maps = []
    for c in range(C):
        m = dict(shards[c])
        m.update(w1=W1, w2=W2, w3=W3, bb=bb)
        in_maps.append(m)

    nc = _build_program(t_blk)
    res = run_bass_kernel_spmd(nc, in_maps, core_ids=list(range(C)), trace=_trace)
    LAST_RESULTS = res

    outs = np.stack([res.results[c]["outT"] for c in range(C)])  # [C, 128, SLOTS] bf16
    out = outs.transpose(0, 2, 1)[node_core, node_col]
    return np.ascontiguousarray(out, dtype=np.float32)

